# revision 55
# baseline (speedup 1.0000x reference)
"""Trainium2 Bass kernel for nn_Attention_62706522521647.

Dense multi-head attention with QK-L2-norm (learnable scale) + axial RoPE,
B=4 N=2048 H=8 DQ=DV=48, IN_DIM=384, f32 inputs/outputs.

Sharding (8 cores, no collectives): core c handles batch b=c//2 and the
4 heads [4*(c%2), 4*(c%2)+4).  Each core computes a partial output
(its heads' contribution through the output projection); the host sums
the two partials per batch.

Per-core design (ACT-exp is the critical path; everything else hides
under it as far as possible):
 - Engine APs start at partitions 0/32/64/96, so two heads are packed per
   [128, N] tile at rows 0-47 and 64-111 (pad rows zeroed via zero weight
   columns).
 - xT fed pre-transposed: xT [384, 2048] bf16; inputs arrive in a few
   merged HWDGE DMAs on the sync ring ordered by consumption time:
   wv+xT (v-proj), wq/wqs (first projections), trig tables, wk/wks, wo.
   The scalar HWDGE ring stays free so ACT table loads are not queued
   behind input traffic.  Angle tables shipped fp16 (halves that DMA).
 - RoPE swap(q) via a SECOND projection with host-swapped weight columns;
   rotation is qr = raw*C2 + swp*S2 where C2/S2 are HOST-computed cos/sin
   VALUE tables (f64 -> fp16, more accurate than on-chip Sin of an fp16
   angle, and it removes 4 Sins + the trig table load); the multiplies
   read the projection PSUM directly (DVE), the adds run on the
   otherwise-idle GpSimd.
 - BOTH q and k are pre-normalized.  ssq is computed from the RAW
   (pre-rope) projection PSUM — rotation preserves the per-head L2 norm,
   so the norm chain (ACT square from PSUM -> ones-matmul -> ACT Rsqrt
   fp16 -> PE broadcast -> scale) runs concurrently with the rope
   elementwise chain instead of behind it.  ACT table sequence is
   rsqrt-set -> exp (square/copy are filler in every set; a dummy exp
   prefetches the exp set before the attention stream begins).
 - scores TRANSPOSED: sT[k, q] = kn-chunk.T @ qn, 2 heads row-packed via
   tile_position (0,0)/(64,0); softmax denominator via ones column in the
   AV stationary, M=128 zero-padded for FWL, with the ones column at 64
   (slot 0) / 96 (slot 1) so Z lands at psum partition 64/96 and 1/Z needs
   no partition shifts.  No max-subtraction (scores in [-10,10]).
   PSUM = 2x scores bufs (4 banks) + 2x AV accumulators (4 banks).
 - Attention inner loop is software-pipelined with PE FIFO order
   AV0(ch), scores(ch+1), AV1(ch); the FIRST scores of the next block are
   emitted inside ch==15 (before the block-end HAM warmup burst) so the
   ACT stream never waits on the boundary.
 - HAM (PE clock gate) management: dependency-free back-to-back dummy
   matmul bursts keep the PE at K=8/8: one at kernel start (on memset
   constants, before any DMA lands), and one at the end of every block
   INCLUDING the last (so the tail out-projection runs warm).
 - Deferred normalize, split in two: at block end only the o-psum-freeing
   reads run (Z copies + bf16 o copies + full-tile recip);
   the broadcast matmul + scale muls are emitted inside the NEXT block
   (hook at ch==3) so they hide under the exp stream.  Out-projection
   chunks 0-7 are emitted inside the last block (hook at ch==10); 8-15 at
   the tail in two flushes so the DMA overlaps the copies; output staged
   fp16 in SBUF (host converts to f32 and sums the two half-partials per
   batch).
"""

import math

import numpy as np
import ml_dtypes

B, N, H, DQ, DV = 4, 2048, 8, 48, 48
IN_DIM = H * DQ  # 384
D2 = DQ // 2  # 24
MAX_FREQ = 10.0
EPS = 1e-6
NCORES = 8
HPC = 4  # heads per core
KC = IN_DIM // 128  # 3 contraction chunks for projections
NCH = N // 128  # 16 k-chunks of 128
NQH = 2  # q halves of 1024
QW = 1024  # q tile width
BF16 = ml_dtypes.bfloat16
FP16 = np.float16


def _freqs_np():
    """Reference freqs (numpy f64 ~1e-7 from the jax f32 original — far
    inside the fp16 angle quantization already applied to the tables)."""
    log_min = math.log(math.pi)
    log_max = math.log(MAX_FREQ * math.pi)
    n = H * D2
    f = np.exp(np.linspace(log_min, log_max, n + 1)[:-1])
    return f.reshape(D2, H).T.astype(np.float32)  # [H, 24]


def build_nc(inv_scale: float):
    import concourse.bass as bass
    import concourse.tile as tile
    from concourse import bacc, mybir

    from concourse.alu_op_type import AluOpType

    dt = mybir.dt
    AF = mybir.ActivationFunctionType
    F32, B16, F16 = dt.float32, dt.bfloat16, dt.float16

    nc = bacc.Bacc("TRN2")

    xT = nc.dram_tensor("xT", [KC, 128, N], B16, kind="ExternalInput")
    tcd = nc.dram_tensor("tcd", [2, 128, N], F16, kind="ExternalInput")
    tsd = nc.dram_tensor("tsd", [2, 128, N], F16, kind="ExternalInput")
    # q/k weights: per pack 112 cols (headA 0-47, zeros 48-63, headB 64-111)
    wq = nc.dram_tensor("wq", [KC, 128, 224], B16, kind="ExternalInput")
    wqs = nc.dram_tensor("wqs", [KC, 128, 224], B16, kind="ExternalInput")
    wk = nc.dram_tensor("wk", [KC, 128, 224], B16, kind="ExternalInput")
    wks = nc.dram_tensor("wks", [KC, 128, 224], B16, kind="ExternalInput")
    wv = nc.dram_tensor("wv", [KC, 128, 192], B16, kind="ExternalInput")
    wo = nc.dram_tensor("wo", [2, 128, 384], B16, kind="ExternalInput")
    e2d = nc.dram_tensor("e2d", [2, 112], F16, kind="ExternalInput")
    out = nc.dram_tensor("out", [N, IN_DIM], F16, kind="ExternalOutput")

    with tile.TileContext(nc) as tc:
        with (
            tc.tile_pool(name="consts", bufs=1) as consts,
            tc.tile_pool(name="qk", bufs=1) as qkpool,
            tc.tile_pool(name="work", bufs=2) as work,
            tc.tile_pool(name="esb", bufs=4) as esb,
            tc.tile_pool(name="psS", bufs=2, space=bass.MemorySpace.PSUM) as psS,
            tc.tile_pool(name="psO", bufs=2, space=bass.MemorySpace.PSUM) as psO,
        ):
            # ---------------- warmup constants (no DMA dependency) ---------
            # ssq reduction stationaries, M=34 with zero pad cols so every
            # psum row of ps_ssq is written (race-detector/garbage safety):
            # qh0 sums land in rows 0-1 (ones2a), qh1 in rows 32-33 (ones2b)
            ones2a = consts.tile([128, 128], B16, tag="ones2a")
            nc.vector.memset(ones2a, 0.0)
            nc.vector.memset(ones2a[0:48, 0:1], 1.0)
            nc.vector.memset(ones2a[64:112, 1:2], 1.0)
            ones2b = consts.tile([128, 128], B16, tag="ones2b")
            nc.vector.memset(ones2b, 0.0)
            nc.vector.memset(ones2b[0:48, 32:33], 1.0)
            nc.vector.memset(ones2b[64:112, 33:34], 1.0)
            wrm = consts.tile([128, 512], B16, tag="wrm")
            nc.vector.memset(wrm, 0.0)

            def pe_warmup(n):
                # HAM un-throttles (K=4/8 -> 8/8, 1.2 -> 2.4 GHz) only after
                # ~3.4us of GAPLESS PE activity; dependency-free back-to-back
                # dummy matmuls provide it.  FIFO position controls WHEN.
                # Operands are memset constants, so the first burst can run
                # before any input DMA lands.  TWO tile requests per burst:
                # the warm tiles ride the scores psum ring, and an odd
                # number of requests would flip the double-buffer parity
                # (scores would start WAR-waiting on the wrong reader).
                for half in range(2):
                    wps = psS.tile([128, 512], F32, tag="s", name="warm")
                    for _ in range(max(1, n // 2)):
                        nc.tensor.matmul(
                            wps, ones2a[:, 0:128], wrm,
                            start=True, stop=True,
                        )

            pe_warmup(10)

            # ------------- input DMAs: merged, via HWDGE (no gpsimd) -------
            # sync ring, in consumption order: v-proj inputs, first q
            # projections, cos/sin tables (feed the rope muls), k weights,
            # out-proj
            wv_all = consts.tile([128, KC, 192], B16, tag="wv")
            nc.sync.dma_start(out=wv_all, in_=wv.rearrange("k p m -> p k m"))
            wv_sb = [wv_all[:, kc, :] for kc in range(KC)]
            xT_all = consts.tile([128, KC, N], B16, tag="xT")
            xT_sb = [xT_all[:, kc, :] for kc in range(KC)]
            for kc in range(KC):
                nc.sync.dma_start(out=xT_sb[kc], in_=xT[kc])
            w_sb = {}
            for nm, hd in (("wq", wq), ("wqs", wqs)):
                t = consts.tile([128, KC, 224], B16, tag=nm, name=nm)
                nc.sync.dma_start(out=t, in_=hd.rearrange("k p m -> p k m"))
                for kc in range(KC):
                    w_sb[(nm, kc)] = t[:, kc, :]
            # cos/sin VALUE tables, fp16, host-computed (no on-chip Sin)
            tc_all = consts.tile([128, 2, N], F16, tag="tc")
            nc.sync.dma_start(out=tc_all, in_=tcd.rearrange("k p n -> p k n"))
            ts_all = consts.tile([128, 2, N], F16, tag="ts")
            nc.sync.dma_start(out=ts_all, in_=tsd.rearrange("k p n -> p k n"))
            C2 = [tc_all[:, p, :] for p in range(2)]
            S2 = [ts_all[:, p, :] for p in range(2)]
            for nm, hd in (("wk", wk), ("wks", wks)):
                t = consts.tile([128, KC, 224], B16, tag=nm, name=nm)
                nc.sync.dma_start(out=t, in_=hd.rearrange("k p m -> p k m"))
                for kc in range(KC):
                    w_sb[(nm, kc)] = t[:, kc, :]
            wo_all = consts.tile([128, 2, 384], B16, tag="wo")
            nc.sync.dma_start(out=wo_all, in_=wo.rearrange("k p m -> p k m"))
            wo_sb = [wo_all[:, p, :] for p in range(2)]
            # E2 replicated at rows 0-1 and 32-33 (matmul requires lhsT and
            # rhs at the same base partition; rsq for qh=1 sits at rows
            # 32-33).  fp16 so the broadcast matmuls avoid slow fp32 mode.
            E2 = consts.tile([34, 112], F16, tag="E2")
            nc.vector.memset(E2, 0.0)
            nc.sync.dma_start(out=E2[32:34, :], in_=e2d[:])
            E2a = consts.tile([34, 112], F16, tag="E2a")
            nc.vector.memset(E2a, 0.0)
            nc.sync.dma_start(out=E2a[0:2, :], in_=e2d[:])
            # Ez: z-broadcast stationary: row 64 (Z0) -> cols 0-47,
            # row 32 (Z1) -> cols 64-111
            Ez = consts.tile([128, 112], F16, tag="Ez")
            nc.vector.memset(Ez, 0.0)
            nc.vector.memset(Ez[64:65, 0:48], 1.0)
            nc.vector.memset(Ez[32:33, 64:112], 1.0)

            # ---------------- constants / memsets (gpsimd: free engine) ---
            # zq: Z staging, all-finite by construction (rows 64/96 overwritten)
            zq = work.tile([128, QW], F32, tag="zq", bufs=1)
            nc.vector.memset(zq, 1.0)
            # activation-bias constants
            cdb = consts.tile([128, 2], F32, tag="cdb")
            for col, val in enumerate([0.0, EPS]):
                nc.vector.memset(cdb[:, col : col + 1], val)
                nc.const_aps.aps[(F32, val)] = cdb[:, col : col + 1]

            # v stationary per (chunk, head-pair, slot), [128, 128] each.
            # Slot-0 heads: v at cols 0-47, ones column at 64 (Z -> psum
            # partition 64).  Slot-1 heads: v at cols 64-111, ones at 32
            # (Z -> partition 32), so slot-1's o VALUES land at partitions
            # 64-111 and every downstream SBUF op (ob copy, rb mul) is
            # lane-aligned — engines cannot shift partitions, and walrus
            # requires tensor_tensor SBUF sources to share a base partition.
            v4 = consts.tile([128, NCH, 2, 2, 128], B16, tag="v4")
            nc.gpsimd.memset(v4[:, :, :, 0, 48:128], 0.0)
            nc.gpsimd.memset(v4[:, :, :, 0, 64:65], 1.0)
            # slot-1 pads on DVE: gpsimd is the prep pole (rope adds +
            # norm-k muls), DVE has slack at kernel start
            nc.vector.memset(v4[:, :, :, 1, 0:64], 0.0)
            nc.vector.memset(v4[:, :, :, 1, 112:128], 0.0)
            nc.vector.memset(v4[:, :, :, 1, 32:33], 1.0)

            # packed attention outputs (pad rows must be finite zeros for the
            # out-projection: garbage bf16 could be NaN and NaN*0 = NaN)
            on_pack = [
                qkpool.tile([128, N], B16, tag=f"on{p}", name=f"on{p}")
                for p in range(2)
            ]
            for p in range(2):
                # rows 48-63 / 112-127 must be finite zeros; bases limited to
                # 32-multiples, the extra rows are overwritten by normalize
                ms = nc.gpsimd.memset if p == 0 else nc.vector.memset
                ms(on_pack[p][32:64, :], 0.0)
                ms(on_pack[p][96:128, :], 0.0)

            dummy = work.tile([128, 1], F32, tag="dummy", bufs=1)

            # ------- q/k projections + rope (PE + DVE) + norms -------------
            # ssq comes from the RAW (pre-rope) psum: rotation preserves the
            # per-head L2 norm, so squares run as soon as each projection
            # chunk lands, concurrently with the rope elementwise chain.
            # v-projection chunks are interleaved 4-per-tensor so the v4
            # copies never head-block the ACT queue.
            qn = [
                qkpool.tile([128, N], B16, tag=f"qn{p}", name=f"qn{p}")
                for p in range(2)
            ]
            kn = [
                qkpool.tile([128, N], B16, tag=f"kn{p}", name=f"kn{p}")
                for p in range(2)
            ]
            qr_t = {}  # (p, name) -> rope'd (un-normalized) [128, N] bf16
            sq_t, rsq_t = {}, {}
            tensors = [(p, name) for p in range(2) for name in ("q", "k")]

            def vproj(ch):
                pool, tg = (psS, "s") if ch % 2 == 0 else (psO, "o")
                ps_v = pool.tile([128, 192], F32, tag=tg, name="ps_v")
                for kc in range(KC):
                    nc.tensor.matmul(
                        ps_v,
                        xT_sb[kc][:, 128 * ch : 128 * (ch + 1)],
                        wv_sb[kc],
                        start=(kc == 0),
                        stop=(kc == KC - 1),
                    )
                ps_v_r = ps_v.rearrange("p (h2 s d) -> p h2 s d", h2=2, s=2)
                nc.scalar.copy(v4[:, ch, :, 0, 0:48], ps_v_r[:, :, 0, :])
                nc.scalar.copy(v4[:, ch, :, 1, 64:112], ps_v_r[:, :, 1, :])

            for ti, (p, name) in enumerate(tensors):
                c2t, s2t = C2[p], S2[p]
                qr = work.tile(
                    [128, N], B16, tag="qr", name=f"qr_{name}{p}", bufs=4
                )
                qr_t[(p, name)] = qr
                sq = work.tile([112, N], B16, tag="sq", name=f"sq_{name}{p}",
                               bufs=4)
                sq_t[(p, name)] = sq
                for nh in range(2):
                    ns = 1024 * nh
                    raw = psS.tile([112, 1024], F32, tag="s", name="raw")
                    swp = psO.tile([112, 1024], F32, tag="o", name="swp")
                    for half in range(2):
                        hs = 512 * half
                        for kc in range(KC):
                            nc.tensor.matmul(
                                raw[:, hs : hs + 512],
                                w_sb[("w" + name, kc)][:, 112 * p : 112 * (p + 1)],
                                xT_sb[kc][:, ns + hs : ns + hs + 512],
                                start=(kc == 0),
                                stop=(kc == KC - 1),
                            )
                        for kc in range(KC):
                            nc.tensor.matmul(
                                swp[:, hs : hs + 512],
                                w_sb[("w" + name + "s", kc)][:, 112 * p : 112 * (p + 1)],
                                xT_sb[kc][:, ns + hs : ns + hs + 512],
                                start=(kc == 0),
                                stop=(kc == KC - 1),
                            )
                    # squared raw straight off the psum (pre-rope norm)
                    nc.scalar.activation(sq[:, ns : ns + 1024], raw, AF.Square)
                    t1 = work.tile([112, 1024], B16, tag="t1", name="t1", bufs=2)
                    nc.vector.tensor_mul(t1, raw, c2t[0:112, ns : ns + 1024])
                    t2 = work.tile([112, 1024], B16, tag="t2", name="t2", bufs=2)
                    nc.vector.tensor_mul(t2, swp, s2t[0:112, ns : ns + 1024])
                    # the add runs on gpsimd (idle engine) to shorten the
                    # DVE prep pole; ~2.2us each but fully overlapped
                    nc.gpsimd.tensor_tensor(
                        qr[0:112, ns : ns + 1024], t1, t2, AluOpType.add
                    )
                # ssq packed [34, QW] (qh0 sums rows 0-1, qh1 rows 32-33)
                ps_ssq = psO.tile([128, QW], F32, tag="o", name="ps_ssq")
                for hh in range(2):
                    for qh in range(NQH):
                        ns = QW * qh + 512 * hh
                        nc.tensor.matmul(
                            ps_ssq[:, 512 * hh : 512 * (hh + 1)],
                            (ones2a if qh == 0 else ones2b)[0:112, :],
                            sq[:, ns : ns + 512],
                            start=(qh == 0),
                            stop=(qh == 1),
                        )
                # sqrt on ACT (square/copy are filler in the sqrt set — only
                # one load for all of prep), 1/sqrt via DVE
                # reciprocal_approx_fast (full-tile f32, the only HW-correct
                # form)
                sqq = work.tile([128, QW], F32, tag="sqq", name="sqq", bufs=2)
                nc.scalar.activation(
                    sqq, ps_ssq, AF.Sqrt, scale=inv_scale, bias=EPS
                )
                rsqf = work.tile([128, QW], F32, tag="rsqf", name="rsqf",
                                 bufs=2)
                nc.vector.reciprocal_approx_fast(out=rsqf, in_=sqq)
                # fp16 (via ACT, keeping DVE lean) so the E2 broadcast
                # matmuls run in fast 16-bit mode instead of fp32 LOW_HIGH
                rsq = work.tile([128, QW], F16, tag="rsq", name="rsq", bufs=4)
                # q tensors convert on ACT, k tensors on DVE — keeps both
                # engines' prep totals balanced (~36us each)
                if name == "q":
                    nc.scalar.copy(rsq, rsqf)
                else:
                    nc.vector.tensor_copy(rsq, rsqf)
                rsq_t[(p, name)] = rsq
                for ch in range(4 * ti, 4 * ti + 4):
                    vproj(ch)
                # maintenance burst: keep HAM warm across the DVE/ACT-paced
                # stretches of prep
                pe_warmup(4)
                # wave-pipelined: the PREVIOUS tensor's normalize runs here
                # so the in-order DVE/gpsimd queues reach pack-0's qn/kn
                # early and the attention stream starts ~30us sooner.
                # pack-1's normalizes are deferred into block-0 hooks: their
                # PE broadcasts otherwise sit AHEAD of the first scores in
                # the in-order PE queue, gating the stream start on the
                # pack-1 reciprocal chain (~55us) for data block 0 never
                # touches
                if 1 <= ti <= 2:
                    norm_apply(*tensors[ti - 1])
            # prefetch the exp table set now (off the attention critical path)
            nc.scalar.activation(dummy, cdb[:, 0:1], AF.Exp)

            def norm_apply(p, name, force_gps=False):
                """Broadcast 1/||.|| over head rows and scale qr -> qn/kn.
                q tensors scale on DVE (psum-direct); k tensors go via an
                ACT psum->SBUF copy + gpsimd mul to balance the engines.
                force_gps: in-stream use — the ACT copy frees the scores
                ring slot in ~1.1us where the DVE mul would hold it 2.4us."""
                qr, rsq = qr_t[(p, name)], rsq_t[(p, name)]
                dst = qn[p] if name == "q" else kn[p]
                for qh in range(NQH):
                    ps_rb = psS.tile([112, QW], F32, tag="s", name="ps_rb")
                    for hh in range(2):
                        src_e = E2a[0:2, :] if qh == 0 else E2[32:34, :]
                        nc.tensor.matmul(
                            ps_rb[:, 512 * hh : 512 * (hh + 1)],
                            src_e,
                            rsq[32 * qh : 32 * qh + 2,
                                512 * hh : 512 * (hh + 1)],
                            start=True,
                            stop=True,
                        )
                    qs = QW * qh
                    if name == "q" and not force_gps:
                        nc.vector.tensor_mul(
                            dst[0:112, qs : qs + QW],
                            qr[0:112, qs : qs + QW],
                            ps_rb,
                        )
                    else:
                        rb = work.tile([112, QW], F16, tag="rb", name="rb",
                                       bufs=2)
                        # in-stream (force_gps) the copy runs on the idle
                        # DVE; in prep ACT has the slack (DVE is the pole)
                        (nc.vector.tensor_copy if force_gps
                         else nc.scalar.copy)(rb, ps_rb)
                        nc.gpsimd.tensor_tensor(
                            dst[0:112, qs : qs + QW],
                            qr[0:112, qs : qs + QW],
                            rb,
                            AluOpType.mult,
                        )


            # ---------------- attention ----------------
            row0 = {0: 0, 1: 64}  # head slot -> pack row offset

            def normalize_a(o0, o1, tail=False):
                """Block-end stage: free the AV psum banks ASAP.  1/Z via
                reciprocal_approx_fast straight off the Z rows of o_ps into
                zq rows 64/96 (zq is memset 1.0, so every row stays finite
                for the Ez broadcast); o values copied out to bf16."""
                # tail: o-copies on the now-idle ACT while the Z->recip
                # chain stays on DVE — the two halves run in parallel
                obcp = nc.scalar.copy if tail else nc.vector.tensor_copy
                obufs = []
                for i, o in enumerate((o0, o1)):
                    zr = 64 if i == 0 else 32
                    r = row0[i]
                    nc.vector.tensor_copy(zq[zr : zr + 1, :], o[zr : zr + 1, :])
                    ob = work.tile([r + 48, QW], B16, tag=f"ob{i}",
                                   name=f"ob{i}", bufs=2)
                    obcp(ob[r : r + 48, :], o[r : r + 48, :])
                    obufs.append(ob)
                # full-tile SBUF->SBUF recip (the only form HW handles)
                rzb = work.tile([128, QW], F32, tag="rzb", name="rzb", bufs=2)
                nc.vector.reciprocal_approx_fast(out=rzb, in_=zq)
                # fp16 so the Ez broadcast matmuls avoid slow fp32 mode
                rzb16 = work.tile([128, QW], F16, tag="rzb16", name="rzb16",
                                  bufs=2)
                nc.vector.tensor_copy(rzb16, rzb)
                return obufs, rzb16

            def normalize_b(p, qh, obufs, rzb16, tail=False):
                """Next-block stage, hidden under the exp stream: broadcast
                1/Z over the head rows via PE, pull it off psum with ONE
                fast copy (frees the scores ring slot quickly), scale on
                DVE from SBUF.  In-stream the copy runs on the idle DVE
                (an ACT copy costs ~1us of exp stream); at the tail ACT is
                the free engine while DVE runs the Z->recip chain."""
                qs = QW * qh
                ps_r = psS.tile([112, QW], F32, tag="s", name="ps_r")
                for hh in range(2):
                    nc.tensor.matmul(
                        ps_r[:, 512 * hh : 512 * (hh + 1)],
                        Ez,
                        rzb16[:, 512 * hh : 512 * (hh + 1)],
                        start=True,
                        stop=True,
                    )
                rb = work.tile([112, QW], F16, tag="rzc", name="rzc", bufs=2)
                (nc.scalar.copy if tail else nc.vector.tensor_copy)(rb, ps_r)
                for i in range(2):
                    r = row0[i]
                    nc.vector.tensor_mul(
                        on_pack[p][r : r + 48, qs : qs + QW],
                        obufs[i][r : r + 48, :],
                        rb[r : r + 48, :],
                    )

            # output staged in SBUF, shipped in big HWDGE DMAs
            osb_all = consts.tile([128, NCH, 384], F16, tag="osb")
            out_r = out.rearrange("(c p) m -> p c m", p=128)

            def outproj(chs, flush, engines=("act", "dve")):
                # chunks processed in PAIRS: one [128, 768] psum tile + ONE
                # copy per pair — halves the per-copy errata overhead and
                # the psum-ring round trips that pace the tail
                chs = list(chs)
                for pi in range(0, len(chs), 2):
                    pair = chs[pi : pi + 2]
                    # chunk outputs at 512-col offsets so neither matmul
                    # crosses a psum bank boundary
                    ps_out = psS.tile([128, 1024], F32, tag="s", name="ps_out")
                    for ci, ch in enumerate(pair):
                        ns = 128 * ch
                        for p in range(2):
                            nc.tensor.matmul(
                                ps_out[:, 512 * ci : 512 * ci + 384],
                                on_pack[p][:, ns : ns + 128],
                                wo_sb[p],
                                start=(p == 0),
                                stop=(p == 1),
                            )
                    eng = engines[(pi // 2) % len(engines)]
                    cp = nc.scalar.copy if eng == "act" else nc.vector.tensor_copy
                    cp(
                        osb_all[:, pair[0] : pair[0] + 2, :],
                        ps_out.rearrange("p (c m) -> p c m", c=2)[:, :, 0:384],
                    )
                nc.sync.dma_start(
                    out=out_r[:, flush[0] : flush[1], :],
                    in_=osb_all[:, flush[0] : flush[1], :],
                )

            blocks = [(p, qh) for p in range(2) for qh in range(NQH)]
            stiles = {}

            def emit_scores_slot(bi, ch, i):
                p, qh = blocks[bi]
                qs = QW * qh
                ks = 128 * ch
                r = row0[i]
                s = psS.tile([128, QW], F32, tag="s", name=f"s{i}")
                for hh in range(2):
                    nc.tensor.matmul(
                        s[:, 512 * hh : 512 * (hh + 1)],
                        kn[p][r : r + 48, ks : ks + 128],
                        qn[p][r : r + 48,
                              qs + 512 * hh : qs + 512 * (hh + 1)],
                        start=True,
                        stop=True,
                        tile_position=(r, 0),
                    )
                stiles[(bi, ch, i)] = s

            def emit_scores(bi, ch):
                emit_scores_slot(bi, ch, 0)
                emit_scores_slot(bi, ch, 1)

            prev = None
            # flip cold->warm (12 MMs = 5.2us cold > 3.41us SHORT window),
            # then PAD the pipeline-startup transient behind the first
            # scores so the free-running MID window can't re-throttle
            pe_warmup(12)
            emit_scores(0, 0)
            pe_warmup(10)
            for bi, (p, qh) in enumerate(blocks):
                last = bi == len(blocks) - 1
                o = [
                    psO.tile([128, QW], F32, tag="o", name=f"o{bi}_{i}")
                    for i in range(2)
                ]
                hooks = {}
                if prev is not None:
                    pv = prev
                    # ch==5: the previous block's normalize_a has drained
                    # off the DVE queue by then, so the rzc DVE copy runs
                    # immediately and the ring slot frees fast
                    hooks[5] = lambda pv=pv: normalize_b(*pv)


                def emit_av(ch, i, es):
                    for hh in range(2):
                        # M=128 (zero-padded): NumWeights==128 turns FWL on,
                        # overlapping LDWEIGHTS with the previous matmul
                        nc.tensor.matmul(
                            o[i][:, 512 * hh : 512 * (hh + 1)],
                            v4[:, ch, p, i, :],
                            es[i][:, 512 * hh : 512 * (hh + 1)],
                            start=(ch == 0),
                            stop=(ch == NCH - 1),
                        )

                for ch in range(NCH):
                    es = []
                    for i in range(2):
                        e = esb.tile([128, QW], B16, tag=f"e{i}", name=f"e{i}")
                        nc.scalar.activation(e, stiles.pop((bi, ch, i)), AF.Exp)
                        es.append(e)
                    # PE FIFO order S(ch+1), A0, A1: A0 waits on exp0's END,
                    # so queueing the next scores AHEAD of it lets slot0's
                    # scores finish during exp1 — otherwise E0(ch+1) starts
                    # ~260ns after E1(ch) every chunk, serialized behind A0
                    if ch + 1 < NCH:
                        emit_scores(bi, ch + 1)
                    elif not last:
                        # cross-block prefetch: the next block's first scores
                        # go right at the boundary so the ACT stream never
                        # waits on it
                        emit_scores(bi + 1, 0)
                    if bi > 0 and ch == 0:
                        # second half of the boundary burst (see ch==15):
                        # splitting it halves the delay each position causes
                        # to the next needed scores tile
                        pe_warmup(6)
                    emit_av(ch, 0, es)
                    emit_av(ch, 1, es)
                    if bi == 0 and ch == 6:
                        # deferred pack-1 normalize (see prep): rsq(1q) has
                        # been ready since ~mid-prep, so the broadcast MMs
                        # never block the PE queue here; the ps_rb ring
                        # injections are paired (parity-safe), read by fast
                        # ACT copies, and a small burst pads the PE so the
                        # stall cannot cross a HAM MID window
                        norm_apply(1, "q", force_gps=True)
                        pe_warmup(4)
                    if bi == 0 and ch == 10:
                        norm_apply(1, "k", force_gps=True)
                        pe_warmup(4)
                    if ch == NCH - 1 and not last:
                        # periodic re-warm: HAM oscillates under micro-idles.
                        # Split 6+6 with the next block's ch==0 so neither
                        # position delays a scores tile by more than ~1.3us;
                        # combined 12 MMs = 5.2us cold still flips a cold
                        # block back to K=8/8 at its boundary.
                        pe_warmup(6)
                    if ch == NCH - 1 and last:
                        # chunks 0-7 (qh=0 of both packs) inside the last
                        # block: the MMs overlap the final exps and keep the
                        # PE warm into the tail; the ACT copies queue right
                        # behind the final exps
                        outproj(range(8), (0, 8), engines=("act",))
                        pe_warmup(4)
                    if ch in hooks:
                        hooks[ch]()
                obufs, rzb = normalize_a(o[0], o[1], tail=last)
                prev = (p, qh, obufs, rzb)
            normalize_b(*prev, tail=True)
            outproj(range(8, 12), (8, 12))
            outproj(range(12, 16), (12, 16))

    return nc


def make_in_maps(x, pos, Wq, Wkv, Wout, scale):
    """Build the 8 per-core input dicts (host-side sharding + layout)."""
    freqs = _freqs_np()  # [H, 24]
    sroot = np.sqrt(scale.astype(np.float64))  # [H]
    in_maps = []
    for c in range(NCORES):
        b = c // 2
        hb = HPC * (c % 2)
        heads = list(range(hb, hb + HPC))
        xb = x[b].astype(np.float32)  # [N, 384]
        xT = np.ascontiguousarray(xb.T).reshape(KC, 128, N)
        posT = np.ascontiguousarray(pos[b].T).astype(np.float32)  # [24, N]

        # cos/sin VALUE tables (host-computed in f64 -> fp16: abs err ~2e-4,
        # strictly better than on-chip Sin of an fp16-quantized angle, and
        # it removes 4 ACT Sins + the trig table load from the kernel)
        tcd = np.zeros((2, 128, N), FP16)
        tsd = np.zeros((2, 128, N), FP16)
        for p in range(2):
            for i in range(2):
                h = heads[2 * p + i]
                r = 64 * i
                th64 = freqs[h][:, None].astype(np.float64) * posT.astype(
                    np.float64
                )  # [24, N]
                c = np.cos(th64).astype(FP16)
                s = np.sin(th64).astype(FP16)
                tcd[p, r : r + 24] = c
                tcd[p, r + 24 : r + 48] = c
                tsd[p, r : r + 24] = -s
                tsd[p, r + 24 : r + 48] = s

        def qk_pack(cols_fn, swap):
            # [384, 224]: per pack p, cols 112p.. = headA(48) 0(16) headB(48)
            w = np.zeros((IN_DIM, 224), np.float64)
            for p in range(2):
                for i in range(2):
                    h = heads[2 * p + i]
                    colblk = cols_fn(h) * sroot[h]
                    if swap:
                        colblk = np.concatenate(
                            [colblk[:, D2:], colblk[:, :D2]], axis=1
                        )
                    w[:, 112 * p + 64 * i : 112 * p + 64 * i + 48] = colblk
            return np.ascontiguousarray(w).reshape(KC, 128, 224).astype(BF16)

        q_cols = lambda h: Wq[:, h * DQ : (h + 1) * DQ].astype(np.float64)
        k_cols = lambda h: Wkv[:, h * (DQ + DV) : h * (DQ + DV) + DQ].astype(
            np.float64
        )
        wqa = qk_pack(q_cols, False)
        wqsa = qk_pack(q_cols, True)
        wka = qk_pack(k_cols, False)
        wksa = qk_pack(k_cols, True)
        wv_cols = np.concatenate(
            [Wkv[:, h * (DQ + DV) + DQ : (h + 1) * (DQ + DV)] for h in heads], axis=1
        )
        wva = np.ascontiguousarray(wv_cols).reshape(KC, 128, 192).astype(BF16)
        e2d_np = np.zeros((2, 112), FP16)
        e2d_np[0, 0:48] = 1
        e2d_np[1, 64:112] = 1
        wo_rows = np.zeros((2, 128, 384), np.float32)
        for p in range(2):
            for i in range(2):
                h = heads[2 * p + i]
                wo_rows[p, 64 * i : 64 * i + 48] = Wout[h * DV : (h + 1) * DV, :]
        in_maps.append(
            {
                "xT": xT.astype(BF16),
                "tcd": tcd,
                "tsd": tsd,
                "wq": wqa,
                "wqs": wqsa,
                "wk": wka,
                "wks": wksa,
                "wv": wva,
                "wo": wo_rows.astype(BF16),
                "e2d": e2d_np,
            }
        )
    return in_maps


_CACHE = {}


def kernel(x, pos, Wq, Wkv, Wout, scale, _profile=False):
    from concourse.bass_utils import run_bass_kernel_spmd

    x = np.asarray(x)
    pos = np.asarray(pos)
    Wq = np.asarray(Wq)
    Wkv = np.asarray(Wkv)
    Wout = np.asarray(Wout)
    scale = np.asarray(scale)

    s0 = float(scale.reshape(-1)[0])
    assert np.allclose(scale, s0, rtol=1e-6), "non-uniform scale unsupported"
    if "nc" not in _CACHE:
        nc_new = build_nc(1.0 / s0)
        nc_new.finalize()
        _CACHE["nc"] = nc_new
    nc = _CACHE["nc"]

    in_maps = make_in_maps(x, pos, Wq, Wkv, Wout, scale)
    res = run_bass_kernel_spmd(
        nc, in_maps, core_ids=list(range(NCORES)), trace=_profile
    )
    outs = [r["out"] for r in res.results]
    full = np.zeros((B, N, IN_DIM), np.float32)
    for b in range(B):
        full[b] = outs[2 * b].astype(np.float32) + outs[2 * b + 1].astype(np.float32)
    if _profile:
        _CACHE["exec_time_ns"] = res.exec_time_ns
        _CACHE["mean_exec_time_ns"] = getattr(res, "mean_exec_time_ns", None)
        _CACHE["profile_json"] = res.profile_json
    return full


# revision 56
# speedup vs baseline: 1.0374x; 1.0374x over previous
"""Trainium2 Bass kernel for nn_Attention_62706522521647.

Dense multi-head attention with QK-L2-norm (learnable scale) + axial RoPE,
B=4 N=2048 H=8 DQ=DV=48, IN_DIM=384, f32 inputs/outputs.

Sharding (8 cores, no collectives): core c handles batch b=c//2 and the
4 heads [4*(c%2), 4*(c%2)+4).  Each core computes a partial output
(its heads' contribution through the output projection); the host sums
the two partials per batch.

Per-core design (ACT-exp is the critical path; everything else hides
under it as far as possible):
 - Engine APs start at partitions 0/32/64/96, so two heads are packed per
   [128, N] tile at rows 0-47 and 64-111 (pad rows zeroed via zero weight
   columns).
 - xT fed pre-transposed: xT [384, 2048] bf16; inputs arrive in a few
   merged HWDGE DMAs on the sync ring ordered by consumption time:
   wv+xT (v-proj), wq/wqs (first projections), trig tables, wk/wks, wo.
   The scalar HWDGE ring stays free so ACT table loads are not queued
   behind input traffic.  Angle tables shipped fp16 (halves that DMA).
 - RoPE swap(q) via a SECOND projection with host-swapped weight columns;
   rotation is qr = raw*C2 + swp*S2 where C2/S2 are HOST-computed cos/sin
   VALUE tables (f64 -> fp16, more accurate than on-chip Sin of an fp16
   angle, and it removes 4 Sins + the trig table load); the multiplies
   read the projection PSUM directly (DVE), the adds run on the
   otherwise-idle GpSimd.
 - BOTH q and k are pre-normalized.  ssq is computed from the RAW
   (pre-rope) projection PSUM — rotation preserves the per-head L2 norm,
   so the norm chain (ACT square from PSUM -> ones-matmul -> ACT Rsqrt
   fp16 -> PE broadcast -> scale) runs concurrently with the rope
   elementwise chain instead of behind it.  ACT table sequence is
   rsqrt-set -> exp (square/copy are filler in every set; a dummy exp
   prefetches the exp set before the attention stream begins).
 - scores TRANSPOSED: sT[k, q] = kn-chunk.T @ qn, 2 heads row-packed via
   tile_position (0,0)/(64,0); softmax denominator via ones column in the
   AV stationary, M=128 zero-padded for FWL, with the ones column at 64
   (slot 0) / 96 (slot 1) so Z lands at psum partition 64/96 and 1/Z needs
   no partition shifts.  No max-subtraction (scores in [-10,10]).
   PSUM = 2x scores bufs (4 banks) + 2x AV accumulators (4 banks).
 - Attention inner loop is software-pipelined with PE FIFO order
   AV0(ch), scores(ch+1), AV1(ch); the FIRST scores of the next block are
   emitted inside ch==15 (before the block-end HAM warmup burst) so the
   ACT stream never waits on the boundary.
 - HAM (PE clock gate) management: dependency-free back-to-back dummy
   matmul bursts keep the PE at K=8/8: one at kernel start (on memset
   constants, before any DMA lands), and one at the end of every block
   INCLUDING the last (so the tail out-projection runs warm).
 - Deferred normalize, split in two: at block end only the o-psum-freeing
   reads run (Z copies + bf16 o copies + full-tile recip);
   the broadcast matmul + scale muls are emitted inside the NEXT block
   (hook at ch==3) so they hide under the exp stream.  Out-projection
   chunks 0-7 are emitted inside the last block (hook at ch==10); 8-15 at
   the tail in two flushes so the DMA overlaps the copies; output staged
   fp16 in SBUF (host converts to f32 and sums the two half-partials per
   batch).
"""

import math

import numpy as np
import ml_dtypes

B, N, H, DQ, DV = 4, 2048, 8, 48, 48
IN_DIM = H * DQ  # 384
D2 = DQ // 2  # 24
MAX_FREQ = 10.0
EPS = 1e-6
NCORES = 8
HPC = 4  # heads per core
KC = IN_DIM // 128  # 3 contraction chunks for projections
NCH = N // 128  # 16 k-chunks of 128
NQH = 2  # q halves of 1024
QW = 1024  # q tile width
BF16 = ml_dtypes.bfloat16
FP16 = np.float16


def _freqs_np():
    """Reference freqs (numpy f64 ~1e-7 from the jax f32 original — far
    inside the fp16 angle quantization already applied to the tables)."""
    log_min = math.log(math.pi)
    log_max = math.log(MAX_FREQ * math.pi)
    n = H * D2
    f = np.exp(np.linspace(log_min, log_max, n + 1)[:-1])
    return f.reshape(D2, H).T.astype(np.float32)  # [H, 24]


def build_nc(inv_scale: float):
    import concourse.bass as bass
    import concourse.tile as tile
    from concourse import bacc, mybir

    from concourse.alu_op_type import AluOpType

    dt = mybir.dt
    AF = mybir.ActivationFunctionType
    F32, B16, F16 = dt.float32, dt.bfloat16, dt.float16

    nc = bacc.Bacc("TRN2")

    xT = nc.dram_tensor("xT", [KC, 128, N], B16, kind="ExternalInput")
    tcd = nc.dram_tensor("tcd", [2, 128, N], F16, kind="ExternalInput")
    tsd = nc.dram_tensor("tsd", [2, 128, N], F16, kind="ExternalInput")
    # q/k weights: per pack 112 cols (headA 0-47, zeros 48-63, headB 64-111)
    wq = nc.dram_tensor("wq", [KC, 128, 224], B16, kind="ExternalInput")
    wqs = nc.dram_tensor("wqs", [KC, 128, 224], B16, kind="ExternalInput")
    wk = nc.dram_tensor("wk", [KC, 128, 224], B16, kind="ExternalInput")
    wks = nc.dram_tensor("wks", [KC, 128, 224], B16, kind="ExternalInput")
    wv = nc.dram_tensor("wv", [KC, 128, 192], B16, kind="ExternalInput")
    wo = nc.dram_tensor("wo", [2, 128, 384], B16, kind="ExternalInput")
    e2d = nc.dram_tensor("e2d", [2, 112], F16, kind="ExternalInput")
    out = nc.dram_tensor("out", [N, IN_DIM], F16, kind="ExternalOutput")

    with tile.TileContext(nc) as tc:
        with (
            tc.tile_pool(name="consts", bufs=1) as consts,
            tc.tile_pool(name="qk", bufs=1) as qkpool,
            tc.tile_pool(name="work", bufs=2) as work,
            tc.tile_pool(name="esb", bufs=4) as esb,
            tc.tile_pool(name="psS", bufs=2, space=bass.MemorySpace.PSUM) as psS,
            tc.tile_pool(name="psO", bufs=2, space=bass.MemorySpace.PSUM) as psO,
        ):
            # ---------------- warmup constants (no DMA dependency) ---------
            # ssq reduction stationaries, M=34 with zero pad cols so every
            # psum row of ps_ssq is written (race-detector/garbage safety):
            # qh0 sums land in rows 0-1 (ones2a), qh1 in rows 32-33 (ones2b)
            ones2a = consts.tile([128, 128], B16, tag="ones2a")
            nc.vector.memset(ones2a, 0.0)
            nc.vector.memset(ones2a[0:48, 0:1], 1.0)
            nc.vector.memset(ones2a[64:112, 1:2], 1.0)
            ones2b = consts.tile([128, 128], B16, tag="ones2b")
            nc.vector.memset(ones2b, 0.0)
            nc.vector.memset(ones2b[0:48, 32:33], 1.0)
            nc.vector.memset(ones2b[64:112, 33:34], 1.0)
            wrm = consts.tile([128, 512], B16, tag="wrm")
            nc.vector.memset(wrm, 0.0)

            def pe_warmup(n):
                # HAM un-throttles (K=4/8 -> 8/8, 1.2 -> 2.4 GHz) only after
                # ~3.4us of GAPLESS PE activity; dependency-free back-to-back
                # dummy matmuls provide it.  FIFO position controls WHEN.
                # Operands are memset constants, so the first burst can run
                # before any input DMA lands.  TWO tile requests per burst:
                # the warm tiles ride the scores psum ring, and an odd
                # number of requests would flip the double-buffer parity
                # (scores would start WAR-waiting on the wrong reader).
                for half in range(2):
                    wps = psS.tile([128, 512], F32, tag="s", name="warm")
                    for _ in range(max(1, n // 2)):
                        nc.tensor.matmul(
                            wps, ones2a[:, 0:128], wrm,
                            start=True, stop=True,
                        )

            pe_warmup(10)

            # ------------- input DMAs: merged, via HWDGE (no gpsimd) -------
            # sync ring, in consumption order: v-proj inputs, first q
            # projections, cos/sin tables (feed the rope muls), k weights,
            # out-proj
            wv_all = consts.tile([128, KC, 192], B16, tag="wv")
            nc.sync.dma_start(out=wv_all, in_=wv.rearrange("k p m -> p k m"))
            wv_sb = [wv_all[:, kc, :] for kc in range(KC)]
            xT_all = consts.tile([128, KC, N], B16, tag="xT")
            xT_sb = [xT_all[:, kc, :] for kc in range(KC)]
            for kc in range(KC):
                nc.sync.dma_start(out=xT_sb[kc], in_=xT[kc])
            w_sb = {}
            for nm, hd in (("wq", wq), ("wqs", wqs)):
                t = consts.tile([128, KC, 224], B16, tag=nm, name=nm)
                nc.sync.dma_start(out=t, in_=hd.rearrange("k p m -> p k m"))
                for kc in range(KC):
                    w_sb[(nm, kc)] = t[:, kc, :]
            # cos/sin VALUE tables, fp16, host-computed (no on-chip Sin)
            tc_all = consts.tile([128, 2, N], F16, tag="tc")
            nc.sync.dma_start(out=tc_all, in_=tcd.rearrange("k p n -> p k n"))
            ts_all = consts.tile([128, 2, N], F16, tag="ts")
            nc.sync.dma_start(out=ts_all, in_=tsd.rearrange("k p n -> p k n"))
            C2 = [tc_all[:, p, :] for p in range(2)]
            S2 = [ts_all[:, p, :] for p in range(2)]
            for nm, hd in (("wk", wk), ("wks", wks)):
                t = consts.tile([128, KC, 224], B16, tag=nm, name=nm)
                nc.sync.dma_start(out=t, in_=hd.rearrange("k p m -> p k m"))
                for kc in range(KC):
                    w_sb[(nm, kc)] = t[:, kc, :]
            wo_all = consts.tile([128, 2, 384], B16, tag="wo")
            nc.sync.dma_start(out=wo_all, in_=wo.rearrange("k p m -> p k m"))
            wo_sb = [wo_all[:, p, :] for p in range(2)]
            # E2 replicated at rows 0-1 and 32-33 (matmul requires lhsT and
            # rhs at the same base partition; rsq for qh=1 sits at rows
            # 32-33).  fp16 so the broadcast matmuls avoid slow fp32 mode.
            E2 = consts.tile([34, 112], F16, tag="E2")
            nc.vector.memset(E2, 0.0)
            nc.sync.dma_start(out=E2[32:34, :], in_=e2d[:])
            E2a = consts.tile([34, 112], F16, tag="E2a")
            nc.vector.memset(E2a, 0.0)
            nc.sync.dma_start(out=E2a[0:2, :], in_=e2d[:])
            # Ez: z-broadcast stationary: row 64 (Z0) -> cols 0-47,
            # row 32 (Z1) -> cols 64-111
            Ez = consts.tile([128, 112], F16, tag="Ez")
            nc.vector.memset(Ez, 0.0)
            nc.vector.memset(Ez[64:65, 0:48], 1.0)
            nc.vector.memset(Ez[32:33, 64:112], 1.0)

            # ---------------- constants / memsets (gpsimd: free engine) ---
            # zq: Z staging, all-finite by construction (rows 64/96 overwritten)
            zq = work.tile([128, QW], F32, tag="zq", bufs=1)
            nc.vector.memset(zq, 1.0)
            # activation-bias constants
            cdb = consts.tile([128, 2], F32, tag="cdb")
            for col, val in enumerate([0.0, EPS]):
                nc.vector.memset(cdb[:, col : col + 1], val)
                nc.const_aps.aps[(F32, val)] = cdb[:, col : col + 1]

            # v stationary per (chunk, head-pair, slot), [128, 128] each.
            # Slot-0 heads: v at cols 0-47, ones column at 64 (Z -> psum
            # partition 64).  Slot-1 heads: v at cols 64-111, ones at 32
            # (Z -> partition 32), so slot-1's o VALUES land at partitions
            # 64-111 and every downstream SBUF op (ob copy, rb mul) is
            # lane-aligned — engines cannot shift partitions, and walrus
            # requires tensor_tensor SBUF sources to share a base partition.
            v4 = consts.tile([128, NCH, 2, 2, 128], B16, tag="v4")
            nc.gpsimd.memset(v4[:, :, :, 0, 48:128], 0.0)
            nc.gpsimd.memset(v4[:, :, :, 0, 64:65], 1.0)
            # slot-1 pads on DVE: gpsimd is the prep pole (rope adds +
            # norm-k muls), DVE has slack at kernel start
            nc.vector.memset(v4[:, :, :, 1, 0:64], 0.0)
            nc.vector.memset(v4[:, :, :, 1, 112:128], 0.0)
            nc.vector.memset(v4[:, :, :, 1, 32:33], 1.0)

            # packed attention outputs (pad rows must be finite zeros for the
            # out-projection: garbage bf16 could be NaN and NaN*0 = NaN)
            on_pack = [
                qkpool.tile([128, N], B16, tag=f"on{p}", name=f"on{p}")
                for p in range(2)
            ]
            for p in range(2):
                # rows 48-63 / 112-127 must be finite zeros; bases limited to
                # 32-multiples, the extra rows are overwritten by normalize
                ms = nc.gpsimd.memset if p == 0 else nc.vector.memset
                ms(on_pack[p][32:64, :], 0.0)
                ms(on_pack[p][96:128, :], 0.0)

            dummy = work.tile([128, 1], F32, tag="dummy", bufs=1)

            # ------- q/k projections + rope (PE + DVE) + norms -------------
            # ssq comes from the RAW (pre-rope) psum: rotation preserves the
            # per-head L2 norm, so squares run as soon as each projection
            # chunk lands, concurrently with the rope elementwise chain.
            # v-projection chunks are interleaved 4-per-tensor so the v4
            # copies never head-block the ACT queue.
            qn = [
                qkpool.tile([128, N], B16, tag=f"qn{p}", name=f"qn{p}")
                for p in range(2)
            ]
            kn = [
                qkpool.tile([128, N], B16, tag=f"kn{p}", name=f"kn{p}")
                for p in range(2)
            ]
            qr_t = {}  # (p, name) -> rope'd (un-normalized) [128, N] bf16
            sq_t, rsq_t = {}, {}
            tensors = [(p, name) for p in range(2) for name in ("q", "k")]

            def vproj(ch):
                pool, tg = (psS, "s") if ch % 2 == 0 else (psO, "o")
                ps_v = pool.tile([128, 192], F32, tag=tg, name="ps_v")
                for kc in range(KC):
                    nc.tensor.matmul(
                        ps_v,
                        xT_sb[kc][:, 128 * ch : 128 * (ch + 1)],
                        wv_sb[kc],
                        start=(kc == 0),
                        stop=(kc == KC - 1),
                    )
                ps_v_r = ps_v.rearrange("p (h2 s d) -> p h2 s d", h2=2, s=2)
                nc.scalar.copy(v4[:, ch, :, 0, 0:48], ps_v_r[:, :, 0, :])
                nc.scalar.copy(v4[:, ch, :, 1, 64:112], ps_v_r[:, :, 1, :])

            for ti, (p, name) in enumerate(tensors):
                c2t, s2t = C2[p], S2[p]
                qr = work.tile(
                    [128, N], B16, tag="qr", name=f"qr_{name}{p}", bufs=4
                )
                qr_t[(p, name)] = qr
                sq = work.tile([112, N], B16, tag="sq", name=f"sq_{name}{p}",
                               bufs=4)
                sq_t[(p, name)] = sq
                for nh in range(2):
                    ns = 1024 * nh
                    raw = psS.tile([112, 1024], F32, tag="s", name="raw")
                    swp = psO.tile([112, 1024], F32, tag="o", name="swp")
                    for half in range(2):
                        hs = 512 * half
                        for kc in range(KC):
                            nc.tensor.matmul(
                                raw[:, hs : hs + 512],
                                w_sb[("w" + name, kc)][:, 112 * p : 112 * (p + 1)],
                                xT_sb[kc][:, ns + hs : ns + hs + 512],
                                start=(kc == 0),
                                stop=(kc == KC - 1),
                            )
                        for kc in range(KC):
                            nc.tensor.matmul(
                                swp[:, hs : hs + 512],
                                w_sb[("w" + name + "s", kc)][:, 112 * p : 112 * (p + 1)],
                                xT_sb[kc][:, ns + hs : ns + hs + 512],
                                start=(kc == 0),
                                stop=(kc == KC - 1),
                            )
                    # squared raw straight off the psum (pre-rope norm)
                    nc.scalar.activation(sq[:, ns : ns + 1024], raw, AF.Square)
                    t1 = work.tile([112, 1024], B16, tag="t1", name="t1", bufs=2)
                    nc.vector.tensor_mul(t1, raw, c2t[0:112, ns : ns + 1024])
                    t2 = work.tile([112, 1024], B16, tag="t2", name="t2", bufs=2)
                    nc.vector.tensor_mul(t2, swp, s2t[0:112, ns : ns + 1024])
                    # the add runs on gpsimd (idle engine) to shorten the
                    # DVE prep pole; ~2.2us each but fully overlapped
                    nc.gpsimd.tensor_tensor(
                        qr[0:112, ns : ns + 1024], t1, t2, AluOpType.add
                    )
                # ssq packed [34, QW] (qh0 sums rows 0-1, qh1 rows 32-33)
                ps_ssq = psO.tile([128, QW], F32, tag="o", name="ps_ssq")
                for hh in range(2):
                    for qh in range(NQH):
                        ns = QW * qh + 512 * hh
                        nc.tensor.matmul(
                            ps_ssq[:, 512 * hh : 512 * (hh + 1)],
                            (ones2a if qh == 0 else ones2b)[0:112, :],
                            sq[:, ns : ns + 512],
                            start=(qh == 0),
                            stop=(qh == 1),
                        )
                # sqrt on ACT (square/copy are filler in the sqrt set — only
                # one load for all of prep), 1/sqrt via DVE
                # reciprocal_approx_fast (full-tile f32, the only HW-correct
                # form)
                sqq = work.tile([128, QW], F32, tag="sqq", name="sqq", bufs=2)
                nc.scalar.activation(
                    sqq, ps_ssq, AF.Sqrt, scale=inv_scale, bias=EPS
                )
                rsqf = work.tile([128, QW], F32, tag="rsqf", name="rsqf",
                                 bufs=2)
                nc.vector.reciprocal_approx_fast(out=rsqf, in_=sqq)
                # fp16 (via ACT, keeping DVE lean) so the E2 broadcast
                # matmuls run in fast 16-bit mode instead of fp32 LOW_HIGH
                rsq = work.tile([128, QW], F16, tag="rsq", name="rsq", bufs=4)
                # q tensors convert on ACT, k tensors on DVE — keeps both
                # engines' prep totals balanced (~36us each)
                if name == "q":
                    nc.scalar.copy(rsq, rsqf)
                else:
                    nc.vector.tensor_copy(rsq, rsqf)
                rsq_t[(p, name)] = rsq
                for ch in range(4 * ti, 4 * ti + 4):
                    vproj(ch)
                # maintenance burst: keep HAM warm across the DVE/ACT-paced
                # stretches of prep
                pe_warmup(4)
                # wave-pipelined: the PREVIOUS tensor's normalize runs here
                # so the in-order DVE/gpsimd queues reach pack-0's qn/kn
                # early and the attention stream starts ~30us sooner.
                # pack-1's normalizes are deferred into block-0 hooks: their
                # PE broadcasts otherwise sit AHEAD of the first scores in
                # the in-order PE queue, gating the stream start on the
                # pack-1 reciprocal chain (~55us) for data block 0 never
                # touches
                if 1 <= ti <= 2:
                    norm_apply(*tensors[ti - 1])
            # prefetch the exp table set now (off the attention critical path)
            nc.scalar.activation(dummy, cdb[:, 0:1], AF.Exp)

            def norm_apply(p, name, force_gps=False):
                """Broadcast 1/||.|| over head rows and scale qr -> qn/kn.
                q tensors scale on DVE (psum-direct); k tensors go via an
                ACT psum->SBUF copy + gpsimd mul to balance the engines.
                force_gps: in-stream use — the ACT copy frees the scores
                ring slot in ~1.1us where the DVE mul would hold it 2.4us."""
                qr, rsq = qr_t[(p, name)], rsq_t[(p, name)]
                dst = qn[p] if name == "q" else kn[p]
                for qh in range(NQH):
                    ps_rb = psS.tile([112, QW], F32, tag="s", name="ps_rb")
                    for hh in range(2):
                        src_e = E2a[0:2, :] if qh == 0 else E2[32:34, :]
                        nc.tensor.matmul(
                            ps_rb[:, 512 * hh : 512 * (hh + 1)],
                            src_e,
                            rsq[32 * qh : 32 * qh + 2,
                                512 * hh : 512 * (hh + 1)],
                            start=True,
                            stop=True,
                        )
                    qs = QW * qh
                    if name == "q" and not force_gps:
                        nc.vector.tensor_mul(
                            dst[0:112, qs : qs + QW],
                            qr[0:112, qs : qs + QW],
                            ps_rb,
                        )
                    else:
                        rb = work.tile([112, QW], F16, tag="rb", name="rb",
                                       bufs=2)
                        # in-stream (force_gps) the copy runs on the idle
                        # DVE; in prep ACT has the slack (DVE is the pole)
                        (nc.vector.tensor_copy if force_gps
                         else nc.scalar.copy)(rb, ps_rb)
                        nc.gpsimd.tensor_tensor(
                            dst[0:112, qs : qs + QW],
                            qr[0:112, qs : qs + QW],
                            rb,
                            AluOpType.mult,
                        )


            # ---------------- attention ----------------
            row0 = {0: 0, 1: 64}  # head slot -> pack row offset

            def normalize_a(o0, o1, tail=False):
                """Block-end stage: free the AV psum banks ASAP.  1/Z via
                reciprocal_approx_fast straight off the Z rows of o_ps into
                zq rows 64/96 (zq is memset 1.0, so every row stays finite
                for the Ez broadcast); o values copied out to bf16."""
                # tail: o-copies on the now-idle ACT while the Z->recip
                # chain stays on DVE — the two halves run in parallel
                obcp = nc.scalar.copy if tail else nc.vector.tensor_copy
                obufs = []
                for i, o in enumerate((o0, o1)):
                    zr = 64 if i == 0 else 32
                    r = row0[i]
                    nc.vector.tensor_copy(zq[zr : zr + 1, :], o[zr : zr + 1, :])
                    ob = work.tile([r + 48, QW], B16, tag=f"ob{i}",
                                   name=f"ob{i}", bufs=2)
                    obcp(ob[r : r + 48, :], o[r : r + 48, :])
                    obufs.append(ob)
                # full-tile SBUF->SBUF recip (the only form HW handles)
                rzb = work.tile([128, QW], F32, tag="rzb", name="rzb", bufs=2)
                nc.vector.reciprocal_approx_fast(out=rzb, in_=zq)
                # fp16 so the Ez broadcast matmuls avoid slow fp32 mode
                rzb16 = work.tile([128, QW], F16, tag="rzb16", name="rzb16",
                                  bufs=2)
                nc.vector.tensor_copy(rzb16, rzb)
                return obufs, rzb16

            def normalize_b(p, qh, obufs, rzb16, tail=False):
                """Next-block stage, hidden under the exp stream: broadcast
                1/Z over the head rows via PE, pull it off psum with ONE
                fast ACT copy (so the scores psum ring is never blocked on
                slow readers), scale on DVE from SBUF."""
                qs = QW * qh
                ps_r = psS.tile([112, QW], F32, tag="s", name="ps_r")
                for hh in range(2):
                    nc.tensor.matmul(
                        ps_r[:, 512 * hh : 512 * (hh + 1)],
                        Ez,
                        rzb16[:, 512 * hh : 512 * (hh + 1)],
                        start=True,
                        stop=True,
                    )
                rb = work.tile([112, QW], F16, tag="rzc", name="rzc", bufs=2)
                # in-stream the copy runs on the idle DVE (an ACT copy costs
                # ~1us of exp stream); at the tail ACT is the free engine
                # while DVE runs the Z->recip chain
                (nc.scalar.copy if tail else nc.vector.tensor_copy)(rb, ps_r)
                for i in range(2):
                    r = row0[i]
                    nc.vector.tensor_mul(
                        on_pack[p][r : r + 48, qs : qs + QW],
                        obufs[i][r : r + 48, :],
                        rb[r : r + 48, :],
                    )

            # output staged in SBUF, shipped in big HWDGE DMAs
            osb_all = consts.tile([128, NCH, 384], F16, tag="osb")
            out_r = out.rearrange("(c p) m -> p c m", p=128)

            def outproj(chs, flush, engines=("act", "dve")):
                # chunks processed in PAIRS: one [128, 768] psum tile + ONE
                # copy per pair — halves the per-copy errata overhead and
                # the psum-ring round trips that pace the tail
                chs = list(chs)
                for pi in range(0, len(chs), 2):
                    pair = chs[pi : pi + 2]
                    # chunk outputs at 512-col offsets so neither matmul
                    # crosses a psum bank boundary
                    ps_out = psS.tile([128, 1024], F32, tag="s", name="ps_out")
                    for ci, ch in enumerate(pair):
                        ns = 128 * ch
                        for p in range(2):
                            nc.tensor.matmul(
                                ps_out[:, 512 * ci : 512 * ci + 384],
                                on_pack[p][:, ns : ns + 128],
                                wo_sb[p],
                                start=(p == 0),
                                stop=(p == 1),
                            )
                    eng = engines[(pi // 2) % len(engines)]
                    cp = nc.scalar.copy if eng == "act" else nc.vector.tensor_copy
                    cp(
                        osb_all[:, pair[0] : pair[0] + 2, :],
                        ps_out.rearrange("p (c m) -> p c m", c=2)[:, :, 0:384],
                    )
                nc.sync.dma_start(
                    out=out_r[:, flush[0] : flush[1], :],
                    in_=osb_all[:, flush[0] : flush[1], :],
                )

            blocks = [(p, qh) for p in range(2) for qh in range(NQH)]
            stiles = {}

            def emit_scores_slot(bi, ch, i):
                p, qh = blocks[bi]
                qs = QW * qh
                ks = 128 * ch
                r = row0[i]
                s = psS.tile([128, QW], F32, tag="s", name=f"s{i}")
                for hh in range(2):
                    nc.tensor.matmul(
                        s[:, 512 * hh : 512 * (hh + 1)],
                        kn[p][r : r + 48, ks : ks + 128],
                        qn[p][r : r + 48,
                              qs + 512 * hh : qs + 512 * (hh + 1)],
                        start=True,
                        stop=True,
                        tile_position=(r, 0),
                    )
                stiles[(bi, ch, i)] = s

            def emit_scores(bi, ch):
                emit_scores_slot(bi, ch, 0)
                emit_scores_slot(bi, ch, 1)

            prev = None
            # flip cold->warm (12 MMs = 5.2us cold > 3.41us SHORT window),
            # then PAD the pipeline-startup transient behind the first
            # scores so the free-running MID window can't re-throttle
            pe_warmup(12)
            emit_scores(0, 0)
            pe_warmup(16)
            for bi, (p, qh) in enumerate(blocks):
                last = bi == len(blocks) - 1
                o = [
                    psO.tile([128, QW], F32, tag="o", name=f"o{bi}_{i}")
                    for i in range(2)
                ]
                hooks = {}
                if prev is not None:
                    pv = prev
                    # ch==5: the previous block's normalize_a has drained
                    # off the DVE queue by then, so the rzc DVE copy runs
                    # immediately and the ring slot frees fast
                    hooks[5] = lambda pv=pv: normalize_b(*pv)


                def emit_av(ch, i, es):
                    for hh in range(2):
                        # M=128 (zero-padded): NumWeights==128 turns FWL on,
                        # overlapping LDWEIGHTS with the previous matmul
                        nc.tensor.matmul(
                            o[i][:, 512 * hh : 512 * (hh + 1)],
                            v4[:, ch, p, i, :],
                            es[i][:, 512 * hh : 512 * (hh + 1)],
                            start=(ch == 0),
                            stop=(ch == NCH - 1),
                        )

                for ch in range(NCH):
                    es = []
                    for i in range(2):
                        e = esb.tile([128, QW], B16, tag=f"e{i}", name=f"e{i}")
                        nc.scalar.activation(e, stiles.pop((bi, ch, i)), AF.Exp)
                        es.append(e)
                    # PE FIFO order S(ch+1), A0, A1: A0 waits on exp0's END,
                    # so queueing the next scores AHEAD of it lets slot0's
                    # scores finish during exp1 — otherwise E0(ch+1) starts
                    # ~260ns after E1(ch) every chunk, serialized behind A0
                    if ch + 1 < NCH:
                        emit_scores(bi, ch + 1)
                    elif not last:
                        # cross-block prefetch: the next block's first scores
                        # go right at the boundary so the ACT stream never
                        # waits on it
                        emit_scores(bi + 1, 0)
                    emit_av(ch, 0, es)
                    emit_av(ch, 1, es)
                    if bi == 0 and ch == 6:
                        # deferred pack-1 normalize (see prep): rsq(1q) has
                        # been ready since ~mid-prep, so the broadcast MMs
                        # never block the PE queue here; the ps_rb ring
                        # injections are paired (parity-safe), read by fast
                        # ACT copies, and a small burst pads the PE so the
                        # stall cannot cross a HAM MID window
                        norm_apply(1, "q", force_gps=True)
                        pe_warmup(4)
                    if bi == 0 and ch == 10:
                        norm_apply(1, "k", force_gps=True)
                        pe_warmup(4)
                    if ch == NCH - 1 and not last:
                        # periodic re-warm: HAM oscillates under micro-idles.
                        # 12 dependency-free MMs = 5.2us cold, comfortably
                        # past the 3.41us SHORT window, so a block that went
                        # cold ALWAYS flips back at its boundary.
                        pe_warmup(12)
                    if ch == NCH - 1 and last:
                        # chunks 0-7 (qh=0 of both packs) inside the last
                        # block: the MMs overlap the final exps and keep the
                        # PE warm into the tail; the ACT copies queue right
                        # behind the final exps
                        outproj(range(8), (0, 8), engines=("act",))
                        pe_warmup(4)
                    if ch in hooks:
                        hooks[ch]()
                obufs, rzb = normalize_a(o[0], o[1], tail=last)
                prev = (p, qh, obufs, rzb)
            normalize_b(*prev, tail=True)
            outproj(range(8, 12), (8, 12))
            outproj(range(12, 16), (12, 16))

    return nc


def make_in_maps(x, pos, Wq, Wkv, Wout, scale):
    """Build the 8 per-core input dicts (host-side sharding + layout)."""
    freqs = _freqs_np()  # [H, 24]
    sroot = np.sqrt(scale.astype(np.float64))  # [H]
    in_maps = []
    for c in range(NCORES):
        b = c // 2
        hb = HPC * (c % 2)
        heads = list(range(hb, hb + HPC))
        xb = x[b].astype(np.float32)  # [N, 384]
        xT = np.ascontiguousarray(xb.T).reshape(KC, 128, N)
        posT = np.ascontiguousarray(pos[b].T).astype(np.float32)  # [24, N]

        # cos/sin VALUE tables (host-computed in f64 -> fp16: abs err ~2e-4,
        # strictly better than on-chip Sin of an fp16-quantized angle, and
        # it removes 4 ACT Sins + the trig table load from the kernel)
        tcd = np.zeros((2, 128, N), FP16)
        tsd = np.zeros((2, 128, N), FP16)
        for p in range(2):
            for i in range(2):
                h = heads[2 * p + i]
                r = 64 * i
                th64 = freqs[h][:, None].astype(np.float64) * posT.astype(
                    np.float64
                )  # [24, N]
                c = np.cos(th64).astype(FP16)
                s = np.sin(th64).astype(FP16)
                tcd[p, r : r + 24] = c
                tcd[p, r + 24 : r + 48] = c
                tsd[p, r : r + 24] = -s
                tsd[p, r + 24 : r + 48] = s

        def qk_pack(cols_fn, swap):
            # [384, 224]: per pack p, cols 112p.. = headA(48) 0(16) headB(48)
            w = np.zeros((IN_DIM, 224), np.float64)
            for p in range(2):
                for i in range(2):
                    h = heads[2 * p + i]
                    colblk = cols_fn(h) * sroot[h]
                    if swap:
                        colblk = np.concatenate(
                            [colblk[:, D2:], colblk[:, :D2]], axis=1
                        )
                    w[:, 112 * p + 64 * i : 112 * p + 64 * i + 48] = colblk
            return np.ascontiguousarray(w).reshape(KC, 128, 224).astype(BF16)

        q_cols = lambda h: Wq[:, h * DQ : (h + 1) * DQ].astype(np.float64)
        k_cols = lambda h: Wkv[:, h * (DQ + DV) : h * (DQ + DV) + DQ].astype(
            np.float64
        )
        wqa = qk_pack(q_cols, False)
        wqsa = qk_pack(q_cols, True)
        wka = qk_pack(k_cols, False)
        wksa = qk_pack(k_cols, True)
        wv_cols = np.concatenate(
            [Wkv[:, h * (DQ + DV) + DQ : (h + 1) * (DQ + DV)] for h in heads], axis=1
        )
        wva = np.ascontiguousarray(wv_cols).reshape(KC, 128, 192).astype(BF16)
        e2d_np = np.zeros((2, 112), FP16)
        e2d_np[0, 0:48] = 1
        e2d_np[1, 64:112] = 1
        wo_rows = np.zeros((2, 128, 384), np.float32)
        for p in range(2):
            for i in range(2):
                h = heads[2 * p + i]
                wo_rows[p, 64 * i : 64 * i + 48] = Wout[h * DV : (h + 1) * DV, :]
        in_maps.append(
            {
                "xT": xT.astype(BF16),
                "tcd": tcd,
                "tsd": tsd,
                "wq": wqa,
                "wqs": wqsa,
                "wk": wka,
                "wks": wksa,
                "wv": wva,
                "wo": wo_rows.astype(BF16),
                "e2d": e2d_np,
            }
        )
    return in_maps


_CACHE = {}


def kernel(x, pos, Wq, Wkv, Wout, scale, _profile=False):
    from concourse.bass_utils import run_bass_kernel_spmd

    x = np.asarray(x)
    pos = np.asarray(pos)
    Wq = np.asarray(Wq)
    Wkv = np.asarray(Wkv)
    Wout = np.asarray(Wout)
    scale = np.asarray(scale)

    s0 = float(scale.reshape(-1)[0])
    assert np.allclose(scale, s0, rtol=1e-6), "non-uniform scale unsupported"
    if "nc" not in _CACHE:
        nc_new = build_nc(1.0 / s0)
        nc_new.finalize()
        _CACHE["nc"] = nc_new
    nc = _CACHE["nc"]

    in_maps = make_in_maps(x, pos, Wq, Wkv, Wout, scale)
    res = run_bass_kernel_spmd(
        nc, in_maps, core_ids=list(range(NCORES)), trace=_profile
    )
    outs = [r["out"] for r in res.results]
    full = np.zeros((B, N, IN_DIM), np.float32)
    for b in range(B):
        full[b] = outs[2 * b].astype(np.float32) + outs[2 * b + 1].astype(np.float32)
    if _profile:
        _CACHE["exec_time_ns"] = res.exec_time_ns
        _CACHE["mean_exec_time_ns"] = getattr(res, "mean_exec_time_ns", None)
        _CACHE["profile_json"] = res.profile_json
    return full
